# revision 52
# baseline (speedup 1.0000x reference)
"""Differential attention (B=2, S=2048, HS=1024, H=16, KV=4, D=64) on 8 trn2 cores.

Sharding: core c = (b, g) with b = c // 4 (data parallel on batch) and
g = c % 4 (tensor parallel over the 4 KV head groups; each core owns the
4 query heads of its group).  Each core computes its 4 heads' normed
attention output and a row-parallel partial of the output projection
(out_pt = (O_heads @ Wo_rows)^T); the host sums the 4 partials per batch.

Device pipeline per core.  S^T scores run in fp32r (tf32-ish); P = exp(S/8)
is written in fp8e4 (e4m3) and the P@[V|ones] accumulation runs as fp8
DoubleRow matmuls over k-tile PAIRS (256-deep contraction, 0.5 cycles/row)
-- the all-ones block rides along in the out partitions so the softmax
denominator is the sum of the SAME quantized P, keeping normalization
self-consistent (measured ~0.4% on the normalized output).

  proj(qt): xT slice -> Q^T/K^T/V^T projections (PE).  RoPE needs
  swap32(q), which would be a cross-partition move; instead rot(Q)^T comes
  from one tiny 128x128 +-1 block-diag matmul (rot2) applied to the SBUF
  copy of Q^T, and RoPE is 3 lane-aligned DVE ops: q*cos + qrot*sin.
  V^T is PE-transposed and packed into fp8 [k, 2, 64|ones] pair tiles.

  attention(qt, j): flash-style causal over k-tile pairs.  S^T[k,q] strips
  via two 64-contraction fp32r matmuls (min width 256 to dodge the <256
  fp32r 4x penalty), exp on ACT into the pair tile's fp8 slot (emitted
  STAGE k-tiles ahead), causal wedge + pre-diagonal zeroing by one gpsimd
  affine_select per map (odd slots use a 256-wide select with base=-128),
  U^T[128,q] += DoubleRow([V|ones]_pair, P_pair).  Epilogue: reciprocal +
  partition-shift DMA (issued on the DVE queue right after its producer) +
  O = U1/r1 - lam*U2/r2 (lam folded into vb, subtract on gpsimd).

  rms(qt): ones-column matmul row-sums of O^2 -> y/64+eps via one DVE
  tensor_scalar, 1/y on DVE, sqrt on ACT (rsqrt = sqrt o recip; 1 ACT pass),
  gpsimd partition-broadcast, subln_w folded into Wo rows on the host.

  wo(qt): partial^T = Wo_rows.T @ O_norm^T -> DVE copy -> DRAM.

Emission is interleaved at attention-head granularity: rms/wo(qt-1) land
between heads 1-2 and proj(qt+1) between heads 2-3, so the PE-only
proj/wo work fills the PE idle while ACT grinds the exp stream, and ACT
never idles during projections.  Dep-free loads go on the SP queue
(sem-waits hold the issuing queue, so loads never sit behind compute).

PSUM: psS pairs [128,1024] double-buffered (4 banks) + psU [128,1024]
single (2) + aux (2).
"""

import math
import sys
from collections import deque

import numpy as np

try:
    import concourse.bass as bass  # noqa: F401
except ImportError:
    sys.path.insert(0, "/opt/trn_rl_repo")

import concourse.bass as bass
import concourse.tile as tile
from concourse import bacc, mybir
from concourse import bass_utils

f32 = mybir.dt.float32
f32r = mybir.dt.float32r
bf16 = mybir.dt.bfloat16
AF = mybir.ActivationFunctionType
ALU = mybir.AluOpType

B, S, HS = 2, 2048, 1024
H, KV, D = 16, 4, 64
NHL = 4            # query heads per core
NQT = 4            # q tiles of 512
QTW = 512
NKT = 16           # k tiles of 128
NHS = 8            # hs tiles of 128
NEG = -1e9
EPS = 1e-5

_prog_cache = {}


def _build_program(lam: float, repeat: int = 1, internal_io: bool = False):
    nc = bacc.Bacc("TRN2", target_bir_lowering=False, debug=False,
                   enable_asserts=False, num_devices=8)

    kin = "Internal" if internal_io else "ExternalInput"
    kout = "Internal" if internal_io else "ExternalOutput"
    xt = nc.dram_tensor("xt", [128, NHS, S], f32r, kind=kin).ap()
    wq = nc.dram_tensor("wq", [128, NHS, 512], f32r, kind=kin).ap()
    wk = nc.dram_tensor("wk", [128, NHS, 128], f32r, kind=kin).ap()
    rot2 = nc.dram_tensor("rot2", [128, 128], f32r, kind=kin).ap()
    wv = nc.dram_tensor("wv", [128, NHS, 64], f32r, kind=kin).ap()
    wo = nc.dram_tensor("wo", [128, 2, HS], f32r, kind=kin).ap()
    cs_t = nc.dram_tensor("cs_t", [128, 2, S], f32, kind=kin).ap()
    idf = nc.dram_tensor("idf", [64, 64], f32, kind=kin).ap()
    ones = nc.dram_tensor("ones", [128, 64], f32r, kind=kin).ap()
    out_pt = nc.dram_tensor("out_pt", [HS, S], f32, kind=kout).ap()
    if internal_io:
        din = nc.dram_tensor("din", [1, 64], f32, kind="ExternalInput").ap()
        dout = nc.dram_tensor("dout", [1, 64], f32, kind="ExternalOutput").ap()

    with tile.TileContext(nc) as tc:
        with tc.tile_pool(name="persist", bufs=1) as pp, \
             tc.tile_pool(name="loc", bufs=2) as loc, \
             tc.tile_pool(name="pwk", bufs=2) as pwk, \
             tc.tile_pool(name="patt", bufs=5) as pa, \
             tc.tile_pool(name="ep", bufs=2) as pe, \
             tc.tile_pool(name="rmsp", bufs=2) as prm, \
             tc.psum_pool(name="ps", bufs=2) as ps_:

            # single batched DMAs (HWDGE issue is ~625ns per DMA and
            # serializes globally, so tile-per-tile loads cost real time);
            # non-proj(0) loads are emitted late so xt wins the engine race
            wqall = pp.tile([128, NHS, 512], f32r, name="wqall", tag="wqall")
            nc.scalar.dma_start(wqall[:], wq[:])
            rot2_sb = pp.tile([128, 128], f32r, name="rot2", tag="rot2")
            nc.scalar.dma_start(rot2_sb[:], rot2[:])
            wkall = pp.tile([128, NHS, 128], f32r, name="wkall", tag="wkall")
            wvall = pp.tile([128, NHS, 64], f32r, name="wvall", tag="wvall")
            ones_sb = pp.tile([128, 64], f32r, name="ones", tag="ones")
            idf_sb = pp.tile([64, 64], f32, name="idf", tag="idf")
            woall = pp.tile([128, 2, HS], f32r, name="woall", tag="woall")
            k_sb = pp.tile([128, S], f32r, name="k", tag="k")
            # bf16 [V | ones] tiles (one per k tile); the r(everse) variants
            # put [ones | V] so odd heads' O lands on partitions 64:128 and
            # the epilogue sub writes opair's upper half directly (no
            # cross-partition shift of the result)
            va = [pp.tile([128, 128], bf16, name=f"va{kt}", tag=f"va{kt}")
                  for kt in range(NKT)]
            vb = [pp.tile([128, 128], bf16, name=f"vb{kt}", tag=f"vb{kt}")
                  for kt in range(NKT)]
            var = [pp.tile([128, 128], bf16, name=f"var{kt}", tag=f"var{kt}")
                   for kt in range(NKT)]
            vbr = [pp.tile([128, 128], bf16, name=f"vbr{kt}", tag=f"vbr{kt}")
                   for kt in range(NKT)]
            eps_sb = pp.tile([1, 1], f32, name="eps", tag="eps")
            nc.vector.memset(eps_sb[:], EPS)

            def emit_late_loads():
                # idf/ones/wk/wv needed within proj(0); wo much later
                nc.sync.dma_start(idf_sb[:], idf[:])
                nc.sync.dma_start(ones_sb[:], ones[:])
                nc.sync.dma_start(wkall[:], wk[:])
                nc.sync.dma_start(wvall[:], wv[:])
                nc.sync.dma_start(woall[:], wo[:])
                for kt in range(NKT):
                    nc.vector.tensor_copy(va[kt][:, 64:128], ones_sb[:])
                    nc.vector.tensor_copy(vb[kt][:, 64:128], ones_sb[:])
                    nc.vector.tensor_copy(var[kt][:, 0:64], ones_sb[:])
                    nc.vector.tensor_copy(vbr[kt][:, 0:64], ones_sb[:])

            def rope_block(qsb, psr, dst, cosq, sinq):
                # dst = qsb * cos + psr * sin  (psr = rot(q) via rot2 matmul;
                # qs first so psr's PSUM bank frees earliest)
                qs = pwk.tile([128, QTW], f32, name="qs", tag="qs")
                nc.vector.tensor_mul(qs[:], psr[:], sinq)
                qc = pwk.tile([128, QTW], f32, name="qc", tag="qc")
                nc.gpsimd.tensor_mul(qc[:], qsb[:], cosq)
                nc.gpsimd.tensor_add(dst, qc[:], qs[:])

            # ---- PE filler queue: proj/wo are chopped into atomic ~1-2us
            # closures and interleaved one-per-k-tile into the attention
            # loops, so the ACT exp stream never drains while PE does
            # projection work, and PE's per-kt exp-paced bubbles get filled.
            fillq = deque()

            def fill_one():
                if fillq:
                    tag, fn = fillq.popleft()
                    fn()
                    return tag
                return None

            def drain_tag(tag):
                # run fillers until the last closure with `tag` has been run
                if not any(t == tag for t, _ in fillq):
                    return
                while True:
                    t = fill_one()
                    if t == tag and not any(x == tag for x, _ in fillq):
                        return

            def drain_all():
                while fillq:
                    fill_one()

            def enqueue_proj(qt, state, crit_first=False):
                # loads issue immediately (max lead time); matmul work is
                # queued as fillers.  Head 0's rot/rope chain goes early --
                # the next q-tile's first S matmuls wait on qloc[0] and k.
                qlo, qhi = qt * QTW, (qt + 1) * QTW
                xtall = pwk.tile([128, NHS, QTW], f32r, name="xtall",
                                 tag="xtall")
                nc.sync.dma_start(xtall[:, 0:4, :], xt[:, 0:4, qlo:qhi])
                nc.sync.dma_start(xtall[:, 4:8, :], xt[:, 4:8, qlo:qhi])
                csq = loc.tile([128, 2, QTW], f32, name="csq", tag="csq")
                nc.sync.dma_start(csq[:], cs_t[:, :, qlo:qhi])
                cosq, sinq = csq[:, 0, :], csq[:, 1, :]
                qsb = [pwk.tile([128, QTW], f32r, name=f"qsb{j}", tag=f"qsb{j}",
                                bufs=1)
                       for j in range(NHL)]
                qloc = [loc.tile([128, QTW], f32r, name=f"q{j}", tag=f"q{j}")
                        for j in range(NHL)]
                state[qt] = qloc

                def mk_psq(j):
                    def go():
                        psq = ps_.tile([128, QTW], f32, name="psq", tag="aux")
                        for hs in range(NHS):
                            nc.tensor.matmul(
                                psq[:], wqall[:, hs, j * 128:(j + 1) * 128],
                                xtall[:, hs, :], start=(hs == 0),
                                stop=(hs == NHS - 1))
                        nc.vector.tensor_copy(qsb[j][:], psq[:])
                    return go

                def mk_rot(j):
                    def go():
                        psqr = ps_.tile([128, QTW], f32, name="psqr", tag="aux")
                        nc.tensor.matmul(psqr[:], rot2_sb[:], qsb[j][:],
                                         start=True, stop=True)
                        rope_block(qsb[j], psqr, qloc[j][:], cosq, sinq)
                    return go

                def go_k():
                    psk = ps_.tile([128, QTW], f32, name="psk", tag="aux")
                    for hs in range(NHS):
                        nc.tensor.matmul(psk[:], wkall[:, hs, :], xtall[:, hs, :],
                                         start=(hs == 0), stop=(hs == NHS - 1))
                    ksb = pwk.tile([128, QTW], f32r, name="ksb", tag="ksb")
                    nc.vector.tensor_copy(ksb[:], psk[:])
                    state[(qt, "ksb")] = ksb

                def go_krot():
                    ksb = state.pop((qt, "ksb"))
                    pskr = ps_.tile([128, QTW], f32, name="pskr", tag="aux")
                    nc.tensor.matmul(pskr[:], rot2_sb[:], ksb[:],
                                     start=True, stop=True)
                    rope_block(ksb, pskr, k_sb[:, qlo:qhi], cosq, sinq)

                def go_v():
                    psv = ps_.tile([64, QTW], f32, name="psv", tag="aux")
                    for hs in range(NHS):
                        nc.tensor.matmul(psv[:], wvall[:, hs, :], xtall[:, hs, :],
                                         start=(hs == 0), stop=(hs == NHS - 1))
                    vtq = loc.tile([64, QTW], f32, name="vtq", tag="vtq")
                    nc.vector.tensor_copy(vtq[:], psv[:])
                    state[(qt, "vtq")] = vtq

                def mk_vt(kk):
                    def go():
                        vtq = state[(qt, "vtq")]
                        kt = 4 * qt + kk
                        psvt = ps_.tile([128, 64], f32, name="psvt", tag="aux")
                        nc.tensor.transpose(psvt[:],
                                            vtq[:, kk * 128:(kk + 1) * 128],
                                            idf_sb[:])
                        nc.vector.tensor_copy(va[kt][:, 0:64], psvt[:])
                        nc.vector.tensor_scalar_mul(vb[kt][:, 0:64], psvt[:],
                                                    lam)
                        nc.vector.tensor_copy(var[kt][:, 64:128], psvt[:])
                        nc.vector.tensor_scalar_mul(vbr[kt][:, 64:128], psvt[:],
                                                    lam)
                    return go

                if crit_first:
                    # everything head 0 needs, then the rest as fillers
                    for c in (mk_psq(0), mk_rot(0), go_k, go_krot, go_v,
                              mk_vt(0), mk_vt(1), mk_vt(2), mk_vt(3)):
                        fillq.append(("crit", c))
                    for j in range(1, NHL):
                        fillq.append(("proj", mk_psq(j)))
                        fillq.append(("proj", mk_rot(j)))
                else:
                    for c in (mk_psq(0), mk_rot(0), go_k, go_krot,
                              mk_psq(1), mk_rot(1), mk_psq(2), mk_rot(2),
                              mk_psq(3), mk_rot(3), go_v,
                              mk_vt(0), mk_vt(1), mk_vt(2), mk_vt(3)):
                        fillq.append(("proj", c))

            def flush_ep(state):
                # epilogue: O^T = U1/r1 - lam*U2/r2  (no PE ops).  Deferred
                # until after the NEXT head's S prefetch so the exp stream
                # never drains while the reciprocal/shift chain runs.  The
                # U/denominator partition halves swap with head parity (va
                # vs var weights) so the sub writes opair's half directly.
                pend = state.pop("pend_ep", None)
                if pend is None:
                    return
                qt, j, psu = pend
                opair = state[(qt, "op")]
                pt = j // 2
                uh, rh = (0, 64) if j % 2 == 0 else (64, 0)
                # two DVE reads drain psu immediately so the next head's U
                # accumulation isn't gated on the rest of this chain
                wri = pe.tile([128, 2 * QTW], f32, name="wri", tag="wri")
                nc.vector.reciprocal(wri[rh:rh + 64, :], psu[rh:rh + 64, :])
                ucp = pe.tile([128, 2 * QTW], f32, name="ucp", tag="ucp",
                              bufs=1)
                nc.vector.tensor_copy(ucp[uh:uh + 64, :], psu[uh:uh + 64, :])
                nc.sync.dma_start(wri[uh:uh + 64, :], wri[rh:rh + 64, :])
                t1 = pe.tile([128, QTW], f32, name="t1", tag="t1", bufs=1)
                nc.vector.tensor_mul(t1[uh:uh + 64, :], ucp[uh:uh + 64, 0:QTW],
                                     wri[uh:uh + 64, 0:QTW])
                t2 = pe.tile([128, QTW], f32, name="t2", tag="t2", bufs=1)
                nc.vector.tensor_mul(t2[uh:uh + 64, :],
                                     ucp[uh:uh + 64, QTW:2 * QTW],
                                     wri[uh:uh + 64, QTW:2 * QTW])
                nc.gpsimd.tensor_sub(opair[pt][uh:uh + 64, :],
                                     t1[uh:uh + 64, :], t2[uh:uh + 64, :])

            def emit_attention_head(qt, j, state):
                qloc = state[qt]
                if (qt, "op") not in state:
                    state[(qt, "op")] = [
                        loc.tile([128, QTW], f32, name=f"op{t}", tag=f"op{t}")
                        for t in range(2)]
                    state[(qt, "on")] = [
                        loc.tile([128, QTW], f32r, name=f"on{t}", tag=f"on{t}")
                        for t in range(2)]
                last_kt = 4 * qt + 3
                psu = ps_.tile([128, 2 * QTW], f32, name="psu", tag="psU",
                               bufs=1)
                p12s = {}

                def emit_s_exp(kt):
                    jd = kt - 4 * qt
                    q0 = 128 * jd if jd >= 0 else 0
                    # fp32r matmuls under 256 wide run at 1/4 rate; pad the
                    # narrow diagonal S strip to 256 (the pad region is never
                    # exp'd nor read by the bf16 U matmul, which uses q0)
                    qm = min(q0, QTW - 256)
                    pss = ps_.tile([128, 2 * QTW], f32, name="pss", tag="psS")
                    nc.tensor.matmul(
                        pss[:, qm:QTW],
                        k_sb[0:64, kt * 128:(kt + 1) * 128],
                        qloc[j][0:64, qm:QTW],
                        start=True, stop=True, skip_group_check=True)
                    nc.tensor.matmul(
                        pss[:, QTW + qm:2 * QTW],
                        k_sb[64:128, kt * 128:(kt + 1) * 128],
                        qloc[j][64:128, qm:QTW],
                        start=True, stop=True, skip_group_check=True)
                    p12 = pa.tile([128, 2 * QTW], bf16, name="p12", tag="p12")
                    nc.scalar.activation(p12[:, q0:2 * QTW], pss[:, q0:2 * QTW],
                                         AF.Exp, scale=0.125)
                    if jd >= 0:
                        for off in (q0, QTW + q0):
                            nc.gpsimd.affine_select(
                                p12[:, off:off + 128], p12[:, off:off + 128],
                                pattern=[[1, 128]], compare_op=ALU.is_ge,
                                fill=0.0, base=0, channel_multiplier=-1)
                    p12s[kt] = p12

                STAGE = 7
                for kt in range(min(STAGE, last_kt + 1)):
                    emit_s_exp(kt)
                    if kt % 2 == 1:
                        fill_one()
                flush_ep(state)
                for kt in range(last_kt + 1):
                    if kt + STAGE <= last_kt:
                        emit_s_exp(kt + STAGE)
                    jd = kt - 4 * qt
                    q0 = 128 * jd if jd >= 0 else 0
                    p12 = p12s.pop(kt)
                    wa = va[kt] if j % 2 == 0 else var[kt]
                    wb = vb[kt] if j % 2 == 0 else vbr[kt]
                    nc.tensor.matmul(
                        psu[:, q0:QTW], wa[:], p12[:, q0:QTW],
                        start=(kt == 0), stop=(kt == last_kt),
                        skip_group_check=True)
                    nc.tensor.matmul(
                        psu[:, QTW + q0:2 * QTW], wb[:],
                        p12[:, QTW + q0:2 * QTW],
                        start=(kt == 0), stop=(kt == last_kt),
                        skip_group_check=True)
                    fill_one()
                state["pend_ep"] = (qt, j, psu)

            def emit_rms(qt, state, pts=None):
                opair = state[(qt, "op")]
                onq = state[(qt, "on")]
                if pts is None:
                    # single pass over all 4 heads: one Ln + one Exp keeps
                    # ACT table churn at 2 loads per q-tile
                    ssqr = prm.tile([1, 4 * QTW], f32, name="ssqr",
                                    tag="rmsrow4")
                    for j in range(NHL):
                        half, pt = (j % 2) * 64, j // 2
                        osq = prm.tile([128, QTW], f32r, name="osq", tag="osq",
                                       bufs=1)
                        nc.vector.tensor_mul(osq[half:half + 64, :],
                                             opair[pt][half:half + 64, :],
                                             opair[pt][half:half + 64, :])
                        psss = ps_.tile([1, QTW], f32, name="psss", tag="aux")
                        nc.tensor.matmul(psss[:], ones_sb[half:half + 64, 0:1],
                                         osq[half:half + 64, :],
                                         start=True, stop=True)
                        nc.vector.tensor_copy(
                            ssqr[0:1, j * QTW:(j + 1) * QTW], psss[:])
                    lnq = prm.tile([1, 4 * QTW], f32, name="lnq", tag="rmsrow4")
                    nc.scalar.activation(lnq[:], ssqr[:], AF.Ln,
                                         scale=1.0 / 64.0,
                                         bias=eps_sb[0:1, 0:1])
                    rmq = prm.tile([1, 4 * QTW], f32, name="rmq", tag="rmsrow4")
                    nc.scalar.activation(rmq[:], lnq[:], AF.Exp, scale=-0.5)
                    for j in range(NHL):
                        half, pt = (j % 2) * 64, j // 2
                        rsb = prm.tile([128, QTW], f32, name="rsb", tag="rsb",
                                       bufs=1)
                        nc.gpsimd.partition_broadcast(
                            rsb[:], rmq[0:1, j * QTW:(j + 1) * QTW])
                        nc.vector.tensor_mul(onq[pt][half:half + 64, :],
                                             opair[pt][half:half + 64, :],
                                             rsb[half:half + 64, :])
                    return
                for pt in pts:
                    ssqr = prm.tile([1, 2 * QTW], f32, name="ssqr",
                                    tag="rmsrow4")
                    for h2 in range(2):
                        half = h2 * 64
                        osq = prm.tile([128, QTW], f32r, name="osq", tag="osq",
                                       bufs=1)
                        nc.vector.tensor_mul(osq[half:half + 64, :],
                                             opair[pt][half:half + 64, :],
                                             opair[pt][half:half + 64, :])
                        psss = ps_.tile([1, QTW], f32, name="psss", tag="aux")
                        nc.tensor.matmul(psss[:], ones_sb[half:half + 64, 0:1],
                                         osq[half:half + 64, :],
                                         start=True, stop=True)
                        nc.vector.tensor_copy(
                            ssqr[0:1, h2 * QTW:(h2 + 1) * QTW], psss[:])
                    # rsqrt via ln/exp
                    lnq = prm.tile([1, 2 * QTW], f32, name="lnq",
                                    tag="rmsrow4")
                    nc.scalar.activation(lnq[:], ssqr[:], AF.Ln,
                                         scale=1.0 / 64.0,
                                         bias=eps_sb[0:1, 0:1])
                    rmq = prm.tile([1, 2 * QTW], f32, name="rmq",
                                    tag="rmsrow4")
                    nc.scalar.activation(rmq[:], lnq[:], AF.Exp, scale=-0.5)
                    for h2 in range(2):
                        half = h2 * 64
                        rsb = prm.tile([128, QTW], f32, name="rsb", tag="rsb",
                                       bufs=1)
                        nc.gpsimd.partition_broadcast(
                            rsb[:], rmq[0:1, h2 * QTW:(h2 + 1) * QTW])
                        nc.vector.tensor_mul(onq[pt][half:half + 64, :],
                                             opair[pt][half:half + 64, :],
                                             rsb[half:half + 64, :])

            def enqueue_wo(qt, state):
                qlo, qhi = qt * QTW, (qt + 1) * QTW
                onq = state[(qt, "on")]

                def mk_wo(oc):
                    def go():
                        psw = ps_.tile([128, QTW], f32, name="psw", tag="aux")
                        nc.tensor.matmul(psw[:],
                                         woall[:, 0, oc * 128:(oc + 1) * 128],
                                         onq[0][:], start=True, stop=False)
                        nc.tensor.matmul(psw[:],
                                         woall[:, 1, oc * 128:(oc + 1) * 128],
                                         onq[1][:], start=False, stop=True)
                        ow = prm.tile([128, QTW], f32, name="ow", tag="ow",
                                      bufs=4)
                        nc.vector.tensor_copy(ow[:], psw[:])
                        nc.sync.dma_start(
                            out_pt[oc * 128:(oc + 1) * 128, qlo:qhi], ow[:])
                    return go

                for oc in range(8):
                    fillq.append(("wo", mk_wo(oc)))

            for rep in range(repeat):
                state = {}
                enqueue_proj(0, state, crit_first=True)
                if rep == 0:
                    emit_late_loads()
                drain_tag("crit")
                for qt in range(NQT):
                    emit_attention_head(qt, 0, state)
                    emit_attention_head(qt, 1, state)
                    if qt > 0:
                        emit_rms(qt - 1, state)
                    if qt < NQT - 1:
                        enqueue_proj(qt + 1, state)
                    if qt > 0:
                        enqueue_wo(qt - 1, state)
                    emit_attention_head(qt, 2, state)
                    if qt == NQT - 1:
                        emit_rms(qt, state, pts=(0,))
                    emit_attention_head(qt, 3, state)
                    drain_tag("proj")
                flush_ep(state)
                drain_all()
                emit_rms(NQT - 1, state, pts=(1,))
                enqueue_wo(NQT - 1, state)
                drain_all()
    if internal_io:
        # tiny external I/O so the PJRT wrapper has something to move
        with tile.TileContext(nc) as tc2:
            with tc2.tile_pool(name="dio", bufs=1) as dp:
                dt_ = dp.tile([1, 64], f32, name="dt_")
                nc.sync.dma_start(dt_[:], din[:])
                nc.sync.dma_start(dout[:], dt_[:])
    nc.compile()
    return nc


def get_program(lam: float, repeat: int = 1, internal_io: bool = False):
    key = (round(float(lam), 9), repeat, internal_io)
    if key not in _prog_cache:
        _prog_cache[key] = _build_program(float(lam), repeat, internal_io)
    return _prog_cache[key]


def _hsplit(w):
    # [128*n, m] row-major -> [128, n, m] so one DMA loads all n hs-tiles
    n = w.shape[0] // 128
    return np.ascontiguousarray(
        w.reshape(n, 128, w.shape[1]).transpose(1, 0, 2))


def _host_inputs(x, rope_cos, rope_sin, Wq, Wk, Wv, Wo, subln_w, lam):
    cos_t = np.ascontiguousarray(np.tile(rope_cos.T, (4, 1))).astype(np.float32)
    sin_t = np.ascontiguousarray(np.tile(rope_sin.T, (4, 1))).astype(np.float32)
    cs_t = np.ascontiguousarray(np.stack([cos_t, sin_t], axis=1))
    idf = np.eye(64, dtype=np.float32)
    ones = np.ones((128, 64), np.float32)
    sub4 = np.tile(subln_w.astype(np.float32), 4)[:, None]
    # rot2[k, d] so that (rot2.T @ q)[d] = -q[d+32] (d<32), q[d-32] (d>=32),
    # block-diag over the two 64-dim head halves (q1 dims | q2 dims)
    r64 = np.zeros((64, 64), np.float32)
    for d in range(32):
        r64[d + 32, d] = -1.0
        r64[d, d + 32] = 1.0
    rot2 = np.zeros((128, 128), np.float32)
    rot2[0:64, 0:64] = r64
    rot2[64:128, 64:128] = r64

    in_maps = []
    for c in range(8):
        b, g = c // 4, c % 4
        xtc = _hsplit(np.ascontiguousarray(x[b].T).astype(np.float32))
        cols = []
        for j in range(NHL):
            h = 4 * g + j
            cols.append(Wq[:, h * 64:(h + 1) * 64])
            cols.append(Wq[:, (H + h) * 64:(H + h + 1) * 64])
        wq_c = _hsplit(np.concatenate(cols, axis=1).astype(np.float32))
        wk_c = _hsplit(np.concatenate(
            [Wk[:, g * 64:(g + 1) * 64], Wk[:, (KV + g) * 64:(KV + g + 1) * 64]],
            axis=1).astype(np.float32))
        wv_c = _hsplit(Wv[:, g * 64:(g + 1) * 64].astype(np.float32))
        wo_c = _hsplit((Wo[g * 256:(g + 1) * 256, :] * sub4).astype(np.float32))
        in_maps.append({
            "xt": xtc, "wq": wq_c, "wk": wk_c, "rot2": rot2,
            "wv": wv_c, "wo": wo_c,
            "cs_t": cs_t, "idf": idf, "ones": ones,
        })
    return in_maps


def _compute_lam(lambda_q1, lambda_k1, lambda_q2, lambda_k2):
    li = 0.8 - 0.6 * math.exp(-0.3)
    l1 = np.exp(np.dot(lambda_q1.astype(np.float32), lambda_k1.astype(np.float32)))
    l2 = np.exp(np.dot(lambda_q2.astype(np.float32), lambda_k2.astype(np.float32)))
    return float(l1 - l2 + li)


def _numpy_reference(x, rope_cos, rope_sin, attention_mask, Wq, Wk, Wv, Wo,
                     lambda_q1, lambda_k1, lambda_q2, lambda_k2, subln_w):
    """Pure-numpy fallback, only used if the mask is not the expected causal one."""
    bsz, seq_len, _ = x.shape

    def rope(t):
        c = np.concatenate([rope_cos, rope_cos], axis=-1)[None, None]
        s = np.concatenate([rope_sin, rope_sin], axis=-1)[None, None]
        t1, t2 = np.split(t, 2, axis=-1)
        rot = np.concatenate([-t2, t1], axis=-1)
        return t * c + rot * s

    q = (x @ Wq).reshape(bsz, seq_len, 2 * H, D)
    q1 = np.transpose(q[:, :, :H], (0, 2, 1, 3))
    q2 = np.transpose(q[:, :, H:], (0, 2, 1, 3))
    k = (x @ Wk).reshape(bsz, seq_len, 2 * KV, D)
    k1 = np.transpose(k[:, :, :KV], (0, 2, 1, 3))
    k2 = np.transpose(k[:, :, KV:], (0, 2, 1, 3))
    v = np.transpose((x @ Wv).reshape(bsz, seq_len, KV, D), (0, 2, 1, 3))
    q1, q2, k1, k2 = rope(q1), rope(q2), rope(k1), rope(k2)
    gr = H // KV
    k1 = np.repeat(k1, gr, axis=1)
    k2 = np.repeat(k2, gr, axis=1)
    v = np.repeat(v, gr, axis=1)
    scale = 1.0 / math.sqrt(D)

    def smax(a):
        a = a - a.max(axis=-1, keepdims=True)
        e = np.exp(a)
        return e / e.sum(axis=-1, keepdims=True)

    a1 = smax(np.einsum("bhqd,bhkd->bhqk", q1, k1) * scale + attention_mask)
    a2 = smax(np.einsum("bhqd,bhkd->bhqk", q2, k2) * scale + attention_mask)
    lam = _compute_lam(lambda_q1, lambda_k1, lambda_q2, lambda_k2)
    attn = a1 - lam * a2
    out = np.einsum("bhqk,bhkd->bhqd", attn, v)
    inv = 1.0 / np.sqrt(np.mean(out * out, axis=-1, keepdims=True) + EPS)
    out = out * inv * subln_w
    out = np.transpose(out, (0, 2, 1, 3)).reshape(bsz, seq_len, HS)
    return (out @ Wo).astype(np.float32)


LAST_RESULT = None


def kernel(x, rope_cos, rope_sin, attention_mask, Wq, Wk, Wv, Wo,
           lambda_q1, lambda_k1, lambda_q2, lambda_k2, subln_w):
    global LAST_RESULT
    x = np.asarray(x, np.float32)
    kk, qq = np.arange(S)[:, None], np.arange(S)[None, :]
    causal = np.where(qq <= kk, 0.0, NEG).astype(np.float32)[None, None]
    am = np.asarray(attention_mask, np.float32)
    if am.shape != (1, 1, S, S) or not np.array_equal(am, causal):
        return _numpy_reference(x, rope_cos, rope_sin, am, Wq, Wk, Wv, Wo,
                                lambda_q1, lambda_k1, lambda_q2, lambda_k2,
                                subln_w)

    lam = _compute_lam(lambda_q1, lambda_k1, lambda_q2, lambda_k2)
    nc = get_program(lam)
    in_maps = _host_inputs(x, np.asarray(rope_cos, np.float32),
                           np.asarray(rope_sin, np.float32),
                           np.asarray(Wq, np.float32), np.asarray(Wk, np.float32),
                           np.asarray(Wv, np.float32), np.asarray(Wo, np.float32),
                           np.asarray(subln_w, np.float32), lam)
    res = bass_utils.run_bass_kernel_spmd(nc, in_maps, core_ids=list(range(8)))
    LAST_RESULT = res
    y = np.zeros((B, S, HS), np.float32)
    for c in range(8):
        y[c // 4] += res.results[c]["out_pt"].T
    return y


# revision 53
# speedup vs baseline: 1.0061x; 1.0061x over previous
"""Differential attention (B=2, S=2048, HS=1024, H=16, KV=4, D=64) on 8 trn2 cores.

Sharding: core c = (b, g) with b = c // 4 (data parallel on batch) and
g = c % 4 (tensor parallel over the 4 KV head groups; each core owns the
4 query heads of its group).  Each core computes its 4 heads' normed
attention output and a row-parallel partial of the output projection
(out_pt = (O_heads @ Wo_rows)^T); the host sums the 4 partials per batch.

Device pipeline per core.  S^T scores run in fp32r (tf32-ish); P = exp(S/8)
is written in fp8e4 (e4m3) and the P@[V|ones] accumulation runs as fp8
DoubleRow matmuls over k-tile PAIRS (256-deep contraction, 0.5 cycles/row)
-- the all-ones block rides along in the out partitions so the softmax
denominator is the sum of the SAME quantized P, keeping normalization
self-consistent (measured ~0.4% on the normalized output).

  proj(qt): xT slice -> Q^T/K^T/V^T projections (PE).  RoPE needs
  swap32(q), which would be a cross-partition move; instead rot(Q)^T comes
  from one tiny 128x128 +-1 block-diag matmul (rot2) applied to the SBUF
  copy of Q^T, and RoPE is 3 lane-aligned DVE ops: q*cos + qrot*sin.
  V^T is PE-transposed and packed into fp8 [k, 2, 64|ones] pair tiles.

  attention(qt, j): flash-style causal over k-tile pairs.  S^T[k,q] strips
  via two 64-contraction fp32r matmuls (min width 256 to dodge the <256
  fp32r 4x penalty), exp on ACT into the pair tile's fp8 slot (emitted
  STAGE k-tiles ahead), causal wedge + pre-diagonal zeroing by one gpsimd
  affine_select per map (odd slots use a 256-wide select with base=-128),
  U^T[128,q] += DoubleRow([V|ones]_pair, P_pair).  Epilogue: reciprocal +
  partition-shift DMA (issued on the DVE queue right after its producer) +
  O = U1/r1 - lam*U2/r2 (lam folded into vb, subtract on gpsimd).

  rms(qt): ones-column matmul row-sums of O^2 -> y/64+eps via one DVE
  tensor_scalar, 1/y on DVE, sqrt on ACT (rsqrt = sqrt o recip; 1 ACT pass),
  gpsimd partition-broadcast, subln_w folded into Wo rows on the host.

  wo(qt): partial^T = Wo_rows.T @ O_norm^T -> DVE copy -> DRAM.

Emission is interleaved at attention-head granularity: rms/wo(qt-1) land
between heads 1-2 and proj(qt+1) between heads 2-3, so the PE-only
proj/wo work fills the PE idle while ACT grinds the exp stream, and ACT
never idles during projections.  Dep-free loads go on the SP queue
(sem-waits hold the issuing queue, so loads never sit behind compute).

PSUM: psS pairs [128,1024] double-buffered (4 banks) + psU [128,1024]
single (2) + aux (2).
"""

import math
import sys
from collections import deque

import numpy as np

try:
    import concourse.bass as bass  # noqa: F401
except ImportError:
    sys.path.insert(0, "/opt/trn_rl_repo")

import concourse.bass as bass
import concourse.tile as tile
from concourse import bacc, mybir
from concourse import bass_utils

f32 = mybir.dt.float32
f32r = mybir.dt.float32r
bf16 = mybir.dt.bfloat16
AF = mybir.ActivationFunctionType
ALU = mybir.AluOpType

B, S, HS = 2, 2048, 1024
H, KV, D = 16, 4, 64
NHL = 4            # query heads per core
NQT = 4            # q tiles of 512
QTW = 512
NKT = 16           # k tiles of 128
NHS = 8            # hs tiles of 128
NEG = -1e9
EPS = 1e-5

_prog_cache = {}


def _build_program(lam: float, repeat: int = 1, internal_io: bool = False):
    nc = bacc.Bacc("TRN2", target_bir_lowering=False, debug=False,
                   enable_asserts=False, num_devices=8)

    kin = "Internal" if internal_io else "ExternalInput"
    kout = "Internal" if internal_io else "ExternalOutput"
    xt = nc.dram_tensor("xt", [128, NHS, S], f32r, kind=kin).ap()
    wq = nc.dram_tensor("wq", [128, NHS, 512], f32r, kind=kin).ap()
    wk = nc.dram_tensor("wk", [128, NHS, 128], f32r, kind=kin).ap()
    rot2 = nc.dram_tensor("rot2", [128, 128], f32r, kind=kin).ap()
    wv = nc.dram_tensor("wv", [128, NHS, 64], f32r, kind=kin).ap()
    wo = nc.dram_tensor("wo", [128, 2, HS], f32r, kind=kin).ap()
    cs_t = nc.dram_tensor("cs_t", [128, 2, S], f32, kind=kin).ap()
    idf = nc.dram_tensor("idf", [64, 64], f32, kind=kin).ap()
    ones = nc.dram_tensor("ones", [128, 64], f32r, kind=kin).ap()
    out_pt = nc.dram_tensor("out_pt", [HS, S], f32, kind=kout).ap()
    if internal_io:
        din = nc.dram_tensor("din", [1, 64], f32, kind="ExternalInput").ap()
        dout = nc.dram_tensor("dout", [1, 64], f32, kind="ExternalOutput").ap()

    with tile.TileContext(nc) as tc:
        with tc.tile_pool(name="persist", bufs=1) as pp, \
             tc.tile_pool(name="loc", bufs=2) as loc, \
             tc.tile_pool(name="pwk", bufs=2) as pwk, \
             tc.tile_pool(name="patt", bufs=5) as pa, \
             tc.tile_pool(name="ep", bufs=2) as pe, \
             tc.tile_pool(name="rmsp", bufs=2) as prm, \
             tc.psum_pool(name="ps", bufs=2) as ps_:

            # single batched DMAs (HWDGE issue is ~625ns per DMA and
            # serializes globally, so tile-per-tile loads cost real time);
            # non-proj(0) loads are emitted late so xt wins the engine race
            wqall = pp.tile([128, NHS, 512], f32r, name="wqall", tag="wqall")
            nc.scalar.dma_start(wqall[:], wq[:])
            rot2_sb = pp.tile([128, 128], f32r, name="rot2", tag="rot2")
            nc.scalar.dma_start(rot2_sb[:], rot2[:])
            wkall = pp.tile([128, NHS, 128], f32r, name="wkall", tag="wkall")
            wvall = pp.tile([128, NHS, 64], f32r, name="wvall", tag="wvall")
            ones_sb = pp.tile([128, 64], f32r, name="ones", tag="ones")
            idf_sb = pp.tile([64, 64], f32, name="idf", tag="idf")
            woall = pp.tile([128, 2, HS], f32r, name="woall", tag="woall")
            k_sb = pp.tile([128, S], f32r, name="k", tag="k")
            # bf16 [V | ones] tiles (one per k tile); the r(everse) variants
            # put [ones | V] so odd heads' O lands on partitions 64:128 and
            # the epilogue sub writes opair's upper half directly (no
            # cross-partition shift of the result)
            va = [pp.tile([128, 128], bf16, name=f"va{kt}", tag=f"va{kt}")
                  for kt in range(NKT)]
            vb = [pp.tile([128, 128], bf16, name=f"vb{kt}", tag=f"vb{kt}")
                  for kt in range(NKT)]
            var = [pp.tile([128, 128], bf16, name=f"var{kt}", tag=f"var{kt}")
                   for kt in range(NKT)]
            vbr = [pp.tile([128, 128], bf16, name=f"vbr{kt}", tag=f"vbr{kt}")
                   for kt in range(NKT)]
            eps_sb = pp.tile([1, 1], f32, name="eps", tag="eps")
            nc.vector.memset(eps_sb[:], EPS)

            def emit_late_loads():
                # idf/ones/wk/wv needed within proj(0); wo much later
                nc.sync.dma_start(idf_sb[:], idf[:])
                nc.sync.dma_start(ones_sb[:], ones[:])
                nc.sync.dma_start(wkall[:], wk[:])
                nc.sync.dma_start(wvall[:], wv[:])
                nc.sync.dma_start(woall[:], wo[:])
                for kt in range(NKT):
                    nc.vector.tensor_copy(va[kt][:, 64:128], ones_sb[:])
                    nc.vector.tensor_copy(vb[kt][:, 64:128], ones_sb[:])
                    nc.vector.tensor_copy(var[kt][:, 0:64], ones_sb[:])
                    nc.vector.tensor_copy(vbr[kt][:, 0:64], ones_sb[:])

            def rope_block(qsb, psr, dst, cosq, sinq):
                # dst = qsb * cos + psr * sin  (psr = rot(q) via rot2 matmul;
                # qs first so psr's PSUM bank frees earliest)
                qs = pwk.tile([128, QTW], f32, name="qs", tag="qs")
                nc.vector.tensor_mul(qs[:], psr[:], sinq)
                qc = pwk.tile([128, QTW], f32, name="qc", tag="qc")
                nc.gpsimd.tensor_mul(qc[:], qsb[:], cosq)
                nc.gpsimd.tensor_add(dst, qc[:], qs[:])

            # ---- PE filler queue: proj/wo are chopped into atomic ~1-2us
            # closures and interleaved one-per-k-tile into the attention
            # loops, so the ACT exp stream never drains while PE does
            # projection work, and PE's per-kt exp-paced bubbles get filled.
            fillq = deque()

            def fill_one():
                if fillq:
                    tag, fn = fillq.popleft()
                    fn()
                    return tag
                return None

            def drain_tag(tag):
                # run fillers until the last closure with `tag` has been run
                if not any(t == tag for t, _ in fillq):
                    return
                while True:
                    t = fill_one()
                    if t == tag and not any(x == tag for x, _ in fillq):
                        return

            def drain_all():
                while fillq:
                    fill_one()

            def enqueue_proj(qt, state, crit_first=False):
                # loads issue immediately (max lead time); matmul work is
                # queued as fillers.  Head 0's rot/rope chain goes early --
                # the next q-tile's first S matmuls wait on qloc[0] and k.
                qlo, qhi = qt * QTW, (qt + 1) * QTW
                xtall = pwk.tile([128, NHS, QTW], f32r, name="xtall",
                                 tag="xtall")
                nc.sync.dma_start(xtall[:, 0:4, :], xt[:, 0:4, qlo:qhi])
                nc.sync.dma_start(xtall[:, 4:8, :], xt[:, 4:8, qlo:qhi])
                csq = loc.tile([128, 2, QTW], f32, name="csq", tag="csq")
                nc.sync.dma_start(csq[:], cs_t[:, :, qlo:qhi])
                cosq, sinq = csq[:, 0, :], csq[:, 1, :]
                qsb = [pwk.tile([128, QTW], f32r, name=f"qsb{j}", tag=f"qsb{j}",
                                bufs=1)
                       for j in range(NHL)]
                qloc = [loc.tile([128, QTW], f32r, name=f"q{j}", tag=f"q{j}")
                        for j in range(NHL)]
                state[qt] = qloc

                def mk_psq(j):
                    def go():
                        psq = ps_.tile([128, QTW], f32, name="psq", tag="aux")
                        for hs in range(NHS):
                            nc.tensor.matmul(
                                psq[:], wqall[:, hs, j * 128:(j + 1) * 128],
                                xtall[:, hs, :], start=(hs == 0),
                                stop=(hs == NHS - 1))
                        nc.vector.tensor_copy(qsb[j][:], psq[:])
                    return go

                def mk_rot(j):
                    def go():
                        psqr = ps_.tile([128, QTW], f32, name="psqr", tag="aux")
                        nc.tensor.matmul(psqr[:], rot2_sb[:], qsb[j][:],
                                         start=True, stop=True)
                        rope_block(qsb[j], psqr, qloc[j][:], cosq, sinq)
                    return go

                def go_k():
                    psk = ps_.tile([128, QTW], f32, name="psk", tag="aux")
                    for hs in range(NHS):
                        nc.tensor.matmul(psk[:], wkall[:, hs, :], xtall[:, hs, :],
                                         start=(hs == 0), stop=(hs == NHS - 1))
                    ksb = pwk.tile([128, QTW], f32r, name="ksb", tag="ksb")
                    nc.vector.tensor_copy(ksb[:], psk[:])
                    state[(qt, "ksb")] = ksb

                def go_krot():
                    ksb = state.pop((qt, "ksb"))
                    pskr = ps_.tile([128, QTW], f32, name="pskr", tag="aux")
                    nc.tensor.matmul(pskr[:], rot2_sb[:], ksb[:],
                                     start=True, stop=True)
                    rope_block(ksb, pskr, k_sb[:, qlo:qhi], cosq, sinq)

                def go_v():
                    psv = ps_.tile([64, QTW], f32, name="psv", tag="aux")
                    for hs in range(NHS):
                        nc.tensor.matmul(psv[:], wvall[:, hs, :], xtall[:, hs, :],
                                         start=(hs == 0), stop=(hs == NHS - 1))
                    vtq = loc.tile([64, QTW], f32, name="vtq", tag="vtq")
                    nc.vector.tensor_copy(vtq[:], psv[:])
                    state[(qt, "vtq")] = vtq

                def mk_vt(kk):
                    def go():
                        vtq = state[(qt, "vtq")]
                        kt = 4 * qt + kk
                        psvt = ps_.tile([128, 64], f32, name="psvt", tag="aux")
                        nc.tensor.transpose(psvt[:],
                                            vtq[:, kk * 128:(kk + 1) * 128],
                                            idf_sb[:])
                        nc.vector.tensor_copy(va[kt][:, 0:64], psvt[:])
                        nc.vector.tensor_scalar_mul(vb[kt][:, 0:64], psvt[:],
                                                    lam)
                        nc.vector.tensor_copy(var[kt][:, 64:128], psvt[:])
                        nc.vector.tensor_scalar_mul(vbr[kt][:, 64:128], psvt[:],
                                                    lam)
                    return go

                if crit_first:
                    # everything head 0 needs, then the rest as fillers
                    for c in (mk_psq(0), mk_rot(0), go_k, go_krot, go_v,
                              mk_vt(0), mk_vt(1), mk_vt(2), mk_vt(3)):
                        fillq.append(("crit", c))
                    for j in range(1, NHL):
                        fillq.append(("proj", mk_psq(j)))
                        fillq.append(("proj", mk_rot(j)))
                else:
                    for c in (mk_psq(0), mk_rot(0), go_k, go_krot,
                              mk_psq(1), mk_rot(1), mk_psq(2), mk_rot(2),
                              mk_psq(3), mk_rot(3), go_v,
                              mk_vt(0), mk_vt(1), mk_vt(2), mk_vt(3)):
                        fillq.append(("proj", c))

            def flush_ep(state):
                # epilogue: O^T = U1/r1 - lam*U2/r2  (no PE ops).  Deferred
                # until after the NEXT head's S prefetch so the exp stream
                # never drains while the reciprocal/shift chain runs.  The
                # U/denominator partition halves swap with head parity (va
                # vs var weights) so the sub writes opair's half directly.
                pend = state.pop("pend_ep", None)
                if pend is None:
                    return
                qt, j, psu = pend
                opair = state[(qt, "op")]
                pt = j // 2
                uh, rh = (0, 64) if j % 2 == 0 else (64, 0)
                # two DVE reads drain psu immediately so the next head's U
                # accumulation isn't gated on the rest of this chain
                wri = pe.tile([128, 2 * QTW], f32, name="wri", tag="wri")
                nc.vector.reciprocal(wri[rh:rh + 64, :], psu[rh:rh + 64, :])
                ucp = pe.tile([128, 2 * QTW], f32, name="ucp", tag="ucp",
                              bufs=1)
                nc.vector.tensor_copy(ucp[uh:uh + 64, :], psu[uh:uh + 64, :])
                nc.sync.dma_start(wri[uh:uh + 64, :], wri[rh:rh + 64, :])
                t1 = pe.tile([128, QTW], f32, name="t1", tag="t1", bufs=1)
                nc.vector.tensor_mul(t1[uh:uh + 64, :], ucp[uh:uh + 64, 0:QTW],
                                     wri[uh:uh + 64, 0:QTW])
                t2 = pe.tile([128, QTW], f32, name="t2", tag="t2", bufs=1)
                nc.vector.tensor_mul(t2[uh:uh + 64, :],
                                     ucp[uh:uh + 64, QTW:2 * QTW],
                                     wri[uh:uh + 64, QTW:2 * QTW])
                nc.gpsimd.tensor_sub(opair[pt][uh:uh + 64, :],
                                     t1[uh:uh + 64, :], t2[uh:uh + 64, :])

            def emit_attention_head(qt, j, state):
                qloc = state[qt]
                if (qt, "op") not in state:
                    state[(qt, "op")] = [
                        loc.tile([128, QTW], f32, name=f"op{t}", tag=f"op{t}")
                        for t in range(2)]
                    state[(qt, "on")] = [
                        loc.tile([128, QTW], f32r, name=f"on{t}", tag=f"on{t}")
                        for t in range(2)]
                last_kt = 4 * qt + 3
                psu = ps_.tile([128, 2 * QTW], f32, name="psu", tag="psU",
                               bufs=1)
                p12s = {}

                def emit_s_exp(kt):
                    jd = kt - 4 * qt
                    q0 = 128 * jd if jd >= 0 else 0
                    # fp32r matmuls under 256 wide run at 1/4 rate; pad the
                    # narrow diagonal S strip to 256 (the pad region is never
                    # exp'd nor read by the bf16 U matmul, which uses q0)
                    qm = min(q0, QTW - 256)
                    pss = ps_.tile([128, 2 * QTW], f32, name="pss", tag="psS")
                    nc.tensor.matmul(
                        pss[:, qm:QTW],
                        k_sb[0:64, kt * 128:(kt + 1) * 128],
                        qloc[j][0:64, qm:QTW],
                        start=True, stop=True, skip_group_check=True)
                    nc.tensor.matmul(
                        pss[:, QTW + qm:2 * QTW],
                        k_sb[64:128, kt * 128:(kt + 1) * 128],
                        qloc[j][64:128, qm:QTW],
                        start=True, stop=True, skip_group_check=True)
                    p12 = pa.tile([128, 2 * QTW], bf16, name="p12", tag="p12")
                    nc.scalar.activation(p12[:, q0:2 * QTW], pss[:, q0:2 * QTW],
                                         AF.Exp, scale=0.125)
                    if jd >= 0:
                        for off in (q0, QTW + q0):
                            nc.gpsimd.affine_select(
                                p12[:, off:off + 128], p12[:, off:off + 128],
                                pattern=[[1, 128]], compare_op=ALU.is_ge,
                                fill=0.0, base=0, channel_multiplier=-1)
                    p12s[kt] = p12

                STAGE = 7
                for kt in range(min(STAGE, last_kt + 1)):
                    emit_s_exp(kt)
                    if kt % 2 == 1:
                        fill_one()
                flush_ep(state)
                for kt in range(last_kt + 1):
                    if kt + STAGE <= last_kt:
                        emit_s_exp(kt + STAGE)
                    jd = kt - 4 * qt
                    q0 = 128 * jd if jd >= 0 else 0
                    p12 = p12s.pop(kt)
                    wa = va[kt] if j % 2 == 0 else var[kt]
                    wb = vb[kt] if j % 2 == 0 else vbr[kt]
                    nc.tensor.matmul(
                        psu[:, q0:QTW], wa[:], p12[:, q0:QTW],
                        start=(kt == 0), stop=(kt == last_kt),
                        skip_group_check=True)
                    nc.tensor.matmul(
                        psu[:, QTW + q0:2 * QTW], wb[:],
                        p12[:, QTW + q0:2 * QTW],
                        start=(kt == 0), stop=(kt == last_kt),
                        skip_group_check=True)
                    fill_one()
                state["pend_ep"] = (qt, j, psu)

            def emit_rms(qt, state, pts=None):
                opair = state[(qt, "op")]
                onq = state[(qt, "on")]
                if pts is None:
                    # single pass over all 4 heads: one Ln + one Exp keeps
                    # ACT table churn at 2 loads per q-tile
                    ssqr = prm.tile([1, 4 * QTW], f32, name="ssqr",
                                    tag="rmsrow4")
                    for j in range(NHL):
                        half, pt = (j % 2) * 64, j // 2
                        osq = prm.tile([128, QTW], f32r, name="osq", tag="osq",
                                       bufs=1)
                        nc.vector.tensor_mul(osq[half:half + 64, :],
                                             opair[pt][half:half + 64, :],
                                             opair[pt][half:half + 64, :])
                        psss = ps_.tile([1, QTW], f32, name="psss", tag="aux")
                        nc.tensor.matmul(psss[:], ones_sb[half:half + 64, 0:1],
                                         osq[half:half + 64, :],
                                         start=True, stop=True)
                        nc.vector.tensor_copy(
                            ssqr[0:1, j * QTW:(j + 1) * QTW], psss[:])
                    lnq = prm.tile([1, 4 * QTW], f32, name="lnq", tag="rmsrow4")
                    nc.scalar.activation(lnq[:], ssqr[:], AF.Ln,
                                         scale=1.0 / 64.0,
                                         bias=eps_sb[0:1, 0:1])
                    rmq = prm.tile([1, 4 * QTW], f32, name="rmq", tag="rmsrow4")
                    nc.scalar.activation(rmq[:], lnq[:], AF.Exp, scale=-0.5)
                    for j in range(NHL):
                        half, pt = (j % 2) * 64, j // 2
                        rsb = prm.tile([128, QTW], f32, name="rsb", tag="rsb",
                                       bufs=1)
                        nc.gpsimd.partition_broadcast(
                            rsb[:], rmq[0:1, j * QTW:(j + 1) * QTW])
                        nc.vector.tensor_mul(onq[pt][half:half + 64, :],
                                             opair[pt][half:half + 64, :],
                                             rsb[half:half + 64, :])
                    return
                for pt in pts:
                    ssqr = prm.tile([1, 2 * QTW], f32, name="ssqr",
                                    tag="rmsrow4")
                    for h2 in range(2):
                        half = h2 * 64
                        osq = prm.tile([128, QTW], f32r, name="osq", tag="osq",
                                       bufs=1)
                        nc.vector.tensor_mul(osq[half:half + 64, :],
                                             opair[pt][half:half + 64, :],
                                             opair[pt][half:half + 64, :])
                        psss = ps_.tile([1, QTW], f32, name="psss", tag="aux")
                        nc.tensor.matmul(psss[:], ones_sb[half:half + 64, 0:1],
                                         osq[half:half + 64, :],
                                         start=True, stop=True)
                        nc.vector.tensor_copy(
                            ssqr[0:1, h2 * QTW:(h2 + 1) * QTW], psss[:])
                    # rsqrt via ln/exp
                    lnq = prm.tile([1, 2 * QTW], f32, name="lnq",
                                    tag="rmsrow4")
                    nc.scalar.activation(lnq[:], ssqr[:], AF.Ln,
                                         scale=1.0 / 64.0,
                                         bias=eps_sb[0:1, 0:1])
                    rmq = prm.tile([1, 2 * QTW], f32, name="rmq",
                                    tag="rmsrow4")
                    nc.scalar.activation(rmq[:], lnq[:], AF.Exp, scale=-0.5)
                    for h2 in range(2):
                        half = h2 * 64
                        rsb = prm.tile([128, QTW], f32, name="rsb", tag="rsb",
                                       bufs=1)
                        nc.gpsimd.partition_broadcast(
                            rsb[:], rmq[0:1, h2 * QTW:(h2 + 1) * QTW])
                        nc.vector.tensor_mul(onq[pt][half:half + 64, :],
                                             opair[pt][half:half + 64, :],
                                             rsb[half:half + 64, :])

            def enqueue_wo(qt, state):
                qlo, qhi = qt * QTW, (qt + 1) * QTW
                onq = state[(qt, "on")]

                def mk_wo(oc):
                    def go():
                        psw = ps_.tile([128, QTW], f32, name="psw", tag="aux")
                        nc.tensor.matmul(psw[:],
                                         woall[:, 0, oc * 128:(oc + 1) * 128],
                                         onq[0][:], start=True, stop=False)
                        nc.tensor.matmul(psw[:],
                                         woall[:, 1, oc * 128:(oc + 1) * 128],
                                         onq[1][:], start=False, stop=True)
                        ow = prm.tile([128, QTW], f32, name="ow", tag="ow",
                                      bufs=4)
                        nc.vector.tensor_copy(ow[:], psw[:])
                        nc.sync.dma_start(
                            out_pt[oc * 128:(oc + 1) * 128, qlo:qhi], ow[:])
                    return go

                for oc in range(8):
                    fillq.append(("wo", mk_wo(oc)))

            for rep in range(repeat):
                state = {}
                enqueue_proj(0, state, crit_first=True)
                if rep == 0:
                    emit_late_loads()
                drain_tag("crit")
                for qt in range(NQT):
                    emit_attention_head(qt, 0, state)
                    emit_attention_head(qt, 1, state)
                    if qt > 0:
                        emit_rms(qt - 1, state)
                    if qt < NQT - 1:
                        enqueue_proj(qt + 1, state)
                    if qt > 0:
                        enqueue_wo(qt - 1, state)
                    emit_attention_head(qt, 2, state)
                    emit_attention_head(qt, 3, state)
                    drain_tag("proj")
                flush_ep(state)
                drain_all()
                emit_rms(NQT - 1, state)
                enqueue_wo(NQT - 1, state)
                drain_all()
    if internal_io:
        # tiny external I/O so the PJRT wrapper has something to move
        with tile.TileContext(nc) as tc2:
            with tc2.tile_pool(name="dio", bufs=1) as dp:
                dt_ = dp.tile([1, 64], f32, name="dt_")
                nc.sync.dma_start(dt_[:], din[:])
                nc.sync.dma_start(dout[:], dt_[:])
    nc.compile()
    return nc


def get_program(lam: float, repeat: int = 1, internal_io: bool = False):
    key = (round(float(lam), 9), repeat, internal_io)
    if key not in _prog_cache:
        _prog_cache[key] = _build_program(float(lam), repeat, internal_io)
    return _prog_cache[key]


def _hsplit(w):
    # [128*n, m] row-major -> [128, n, m] so one DMA loads all n hs-tiles
    n = w.shape[0] // 128
    return np.ascontiguousarray(
        w.reshape(n, 128, w.shape[1]).transpose(1, 0, 2))


def _host_inputs(x, rope_cos, rope_sin, Wq, Wk, Wv, Wo, subln_w, lam):
    cos_t = np.ascontiguousarray(np.tile(rope_cos.T, (4, 1))).astype(np.float32)
    sin_t = np.ascontiguousarray(np.tile(rope_sin.T, (4, 1))).astype(np.float32)
    cs_t = np.ascontiguousarray(np.stack([cos_t, sin_t], axis=1))
    idf = np.eye(64, dtype=np.float32)
    ones = np.ones((128, 64), np.float32)
    sub4 = np.tile(subln_w.astype(np.float32), 4)[:, None]
    # rot2[k, d] so that (rot2.T @ q)[d] = -q[d+32] (d<32), q[d-32] (d>=32),
    # block-diag over the two 64-dim head halves (q1 dims | q2 dims)
    r64 = np.zeros((64, 64), np.float32)
    for d in range(32):
        r64[d + 32, d] = -1.0
        r64[d, d + 32] = 1.0
    rot2 = np.zeros((128, 128), np.float32)
    rot2[0:64, 0:64] = r64
    rot2[64:128, 64:128] = r64

    in_maps = []
    for c in range(8):
        b, g = c // 4, c % 4
        xtc = _hsplit(np.ascontiguousarray(x[b].T).astype(np.float32))
        cols = []
        for j in range(NHL):
            h = 4 * g + j
            cols.append(Wq[:, h * 64:(h + 1) * 64])
            cols.append(Wq[:, (H + h) * 64:(H + h + 1) * 64])
        wq_c = _hsplit(np.concatenate(cols, axis=1).astype(np.float32))
        wk_c = _hsplit(np.concatenate(
            [Wk[:, g * 64:(g + 1) * 64], Wk[:, (KV + g) * 64:(KV + g + 1) * 64]],
            axis=1).astype(np.float32))
        wv_c = _hsplit(Wv[:, g * 64:(g + 1) * 64].astype(np.float32))
        wo_c = _hsplit((Wo[g * 256:(g + 1) * 256, :] * sub4).astype(np.float32))
        in_maps.append({
            "xt": xtc, "wq": wq_c, "wk": wk_c, "rot2": rot2,
            "wv": wv_c, "wo": wo_c,
            "cs_t": cs_t, "idf": idf, "ones": ones,
        })
    return in_maps


def _compute_lam(lambda_q1, lambda_k1, lambda_q2, lambda_k2):
    li = 0.8 - 0.6 * math.exp(-0.3)
    l1 = np.exp(np.dot(lambda_q1.astype(np.float32), lambda_k1.astype(np.float32)))
    l2 = np.exp(np.dot(lambda_q2.astype(np.float32), lambda_k2.astype(np.float32)))
    return float(l1 - l2 + li)


def _numpy_reference(x, rope_cos, rope_sin, attention_mask, Wq, Wk, Wv, Wo,
                     lambda_q1, lambda_k1, lambda_q2, lambda_k2, subln_w):
    """Pure-numpy fallback, only used if the mask is not the expected causal one."""
    bsz, seq_len, _ = x.shape

    def rope(t):
        c = np.concatenate([rope_cos, rope_cos], axis=-1)[None, None]
        s = np.concatenate([rope_sin, rope_sin], axis=-1)[None, None]
        t1, t2 = np.split(t, 2, axis=-1)
        rot = np.concatenate([-t2, t1], axis=-1)
        return t * c + rot * s

    q = (x @ Wq).reshape(bsz, seq_len, 2 * H, D)
    q1 = np.transpose(q[:, :, :H], (0, 2, 1, 3))
    q2 = np.transpose(q[:, :, H:], (0, 2, 1, 3))
    k = (x @ Wk).reshape(bsz, seq_len, 2 * KV, D)
    k1 = np.transpose(k[:, :, :KV], (0, 2, 1, 3))
    k2 = np.transpose(k[:, :, KV:], (0, 2, 1, 3))
    v = np.transpose((x @ Wv).reshape(bsz, seq_len, KV, D), (0, 2, 1, 3))
    q1, q2, k1, k2 = rope(q1), rope(q2), rope(k1), rope(k2)
    gr = H // KV
    k1 = np.repeat(k1, gr, axis=1)
    k2 = np.repeat(k2, gr, axis=1)
    v = np.repeat(v, gr, axis=1)
    scale = 1.0 / math.sqrt(D)

    def smax(a):
        a = a - a.max(axis=-1, keepdims=True)
        e = np.exp(a)
        return e / e.sum(axis=-1, keepdims=True)

    a1 = smax(np.einsum("bhqd,bhkd->bhqk", q1, k1) * scale + attention_mask)
    a2 = smax(np.einsum("bhqd,bhkd->bhqk", q2, k2) * scale + attention_mask)
    lam = _compute_lam(lambda_q1, lambda_k1, lambda_q2, lambda_k2)
    attn = a1 - lam * a2
    out = np.einsum("bhqk,bhkd->bhqd", attn, v)
    inv = 1.0 / np.sqrt(np.mean(out * out, axis=-1, keepdims=True) + EPS)
    out = out * inv * subln_w
    out = np.transpose(out, (0, 2, 1, 3)).reshape(bsz, seq_len, HS)
    return (out @ Wo).astype(np.float32)


LAST_RESULT = None


def kernel(x, rope_cos, rope_sin, attention_mask, Wq, Wk, Wv, Wo,
           lambda_q1, lambda_k1, lambda_q2, lambda_k2, subln_w):
    global LAST_RESULT
    x = np.asarray(x, np.float32)
    kk, qq = np.arange(S)[:, None], np.arange(S)[None, :]
    causal = np.where(qq <= kk, 0.0, NEG).astype(np.float32)[None, None]
    am = np.asarray(attention_mask, np.float32)
    if am.shape != (1, 1, S, S) or not np.array_equal(am, causal):
        return _numpy_reference(x, rope_cos, rope_sin, am, Wq, Wk, Wv, Wo,
                                lambda_q1, lambda_k1, lambda_q2, lambda_k2,
                                subln_w)

    lam = _compute_lam(lambda_q1, lambda_k1, lambda_q2, lambda_k2)
    nc = get_program(lam)
    in_maps = _host_inputs(x, np.asarray(rope_cos, np.float32),
                           np.asarray(rope_sin, np.float32),
                           np.asarray(Wq, np.float32), np.asarray(Wk, np.float32),
                           np.asarray(Wv, np.float32), np.asarray(Wo, np.float32),
                           np.asarray(subln_w, np.float32), lam)
    res = bass_utils.run_bass_kernel_spmd(nc, in_maps, core_ids=list(range(8)))
    LAST_RESULT = res
    y = np.zeros((B, S, HS), np.float32)
    for c in range(8):
        y[c // 4] += res.results[c]["out_pt"].T
    return y
